# revision 51
# baseline (speedup 1.0000x reference)
"""Trainium2 Bass kernel for per-position FC decoder stack.

out[b, o3, p] = W3[p] @ (W2[p] @ (W1[p] @ glf[b] + b1[p]) + b2[p]) + b3[p]

All layers are linear, so we fold the ENTIRE stack into a per-position affine
map Meff[p] = W3[p]@W2[p]@W1[p] ([3, 512]) and beff[p] = W3@(W2@b1+b2)+b3
([3]).  MeffT is computed directly (no W1 transpose) by using W1 natural-
layout 128-row chunks as the stationary operand against a zero-padded
block-diagonal W23^T band (moving, 256-wide windows so float32r runs at full
PE rate).  Stage 2 applies MeffT to glf^T (4 accumulating f32r matmuls + one
bias-row matmul) producing the output block [B, (o3, p)] straight in PSUM.

W23 = W3@W2 is itself computed on the PE with a small block-diag W3^T band.
beff rows come from tiny accumulating matmuls against a narrow W23^T band.

Sharding: positions (2048) split across 8 cores; glf replicated.  W1 per core
(16 MiB) is the DMA roofline; all W1 tiles stay resident in SBUF so the DMA
stream never stalls on compute, and per-block output DMAs keep the tail short.
"""

import sys

if "/opt/trn_rl_repo" not in sys.path:
    sys.path.insert(0, "/opt/trn_rl_repo")

import numpy as np

# Problem constants (hardcoded per contest contract)
P_FULL = 2048
NCORES = 8
PP = P_FULL // NCORES  # 256 positions per core
B = 32
I = 512
O1 = 32
O2 = 8
O3 = 3
NT = 16    # W1 tiles of [128, 4*512] (16 positions each)
NBLK = 8   # blocks of 32 positions
POS = 32   # positions per block
NU = 8     # u-groups (4 positions) per block
WIN = 256  # band window width per u-group

_CACHE = {}


def _build_nc():
    import concourse.bass as bass
    import concourse.mybir as mybir
    import concourse.tile as tile
    from concourse import bacc
    from concourse.masks import make_identity

    F32 = mybir.dt.float32
    F32R = mybir.dt.float32r
    ADD = mybir.AluOpType.add

    nc = bacc.Bacc(
        "TRN2", target_bir_lowering=False, debug=False, num_devices=NCORES
    )
    W1 = nc.declare_dram_parameter("W1", [PP, O1, I], F32R, isOutput=False)
    b1 = nc.declare_dram_parameter("b1", [PP, O1], F32R, isOutput=False)
    W2 = nc.declare_dram_parameter("W2", [PP, O2, O1], F32, isOutput=False)
    b2 = nc.declare_dram_parameter("b2", [PP, O2], F32R, isOutput=False)
    W3 = nc.declare_dram_parameter("W3", [PP, O3, O2], F32, isOutput=False)
    b3 = nc.declare_dram_parameter("b3", [PP, O3], F32, isOutput=False)
    glf = nc.declare_dram_parameter("glf", [B, I], F32, isOutput=False)
    out = nc.declare_dram_parameter("out", [B, O3, PP], F32, isOutput=True)

    with tile.TileContext(nc) as tc:
        with (
            tc.tile_pool(name="persist", bufs=1) as pp,
            tc.tile_pool(name="w1s", bufs=NT) as w1p,
            tc.tile_pool(name="mt", bufs=8) as mtp,
            tc.tile_pool(name="pst", bufs=5, space="PSUM") as pstp,
            tc.tile_pool(name="psy", bufs=1, space="PSUM") as psyp,
            tc.tile_pool(name="psb", bufs=2, space="PSUM") as psbp,
        ):
            # ---------------- prep DMAs ----------------
            # scalar queue: small inputs (positions on partitions where used
            # by the DVE-side W23 = W3@W2 computation)
            # w3p[q, h, (x, o2)] = W3[128h + q, x, o2]
            w3p = pp.tile([128, 2 * O3 * O2], F32, tag="w3p")  # [128, 48]
            nc.scalar.dma_start(
                out=w3p[:].rearrange("q (h c) -> q h c", h=2),
                in_=W3[:].rearrange("p x o -> p (x o)").rearrange(
                    "(h q) c -> q h c", q=128
                ),
            )
            # W2p[q, h, (o2, o1)] = W2[128h + q, o2, o1]
            W2p = pp.tile([128, 2 * O2 * O1], F32, tag="W2p")  # [128, 512]
            nc.sync.dma_start(
                out=W2p[:].rearrange("q (h c) -> q h c", h=2),
                in_=W2[:].rearrange("p o i -> p (o i)").rearrange(
                    "(h q) c -> q h c", q=128
                ),
            )
            # b2p[q, h, o2] = b2[128h + q, o2]
            b2p = pp.tile([128, 2 * O2], F32, tag="b2p")  # [128, 16]
            nc.scalar.dma_start(
                out=b2p[:].rearrange("q (h o) -> q h o", h=2),
                in_=b2[:].bitcast(F32).rearrange("(h q) o -> q h o", q=128),
            )
            b1nat = pp.tile([128, 2 * O1], F32, tag="b1nat")
            nc.scalar.dma_start(
                out=b1nat[:].rearrange("q (h o) -> q h o", h=2),
                in_=b1[:].bitcast(F32).rearrange("(h q) o -> q h o", q=128),
            )
            glf_sb = pp.tile([B, I], F32, tag="glf")
            nc.scalar.dma_start(out=glf_sb, in_=glf[:])
            # b3 flat; remapped to block layout inside the basebias add
            b3nat = pp.tile([1, PP * O3], F32, tag="b3nat")  # [1, 768]
            nc.scalar.dma_start(
                out=b3nat,
                in_=b3[:].rearrange("p x -> (p x)").rearrange("(o f) -> o f", o=1),
            )

            # ---------------- W1 stream (all tiles resident) ----------------
            w1tiles = {}
            for t in range(NT):
                w1t = w1p.tile([128, 4 * I], F32R, tag="w1t", name=f"w1t{t}")
                w1tiles[t] = w1t
                w1src = (
                    W1[:]
                    .rearrange("p o i -> (p o) i")[512 * t : 512 * (t + 1), :]
                    .rearrange("(u q) i -> q u i", q=128)
                )
                w1dst = w1t[:].rearrange("q (u i) -> q u i", u=4)
                if t >= NT - 2:
                    # split the last tiles by i-chunk so the final c-chains
                    # can start before the full tile lands
                    for ic in range(4):
                        nc.sync.dma_start(
                            out=w1dst[:, :, 128 * ic : 128 * (ic + 1)],
                            in_=w1src[:, :, 128 * ic : 128 * (ic + 1)],
                        )
                else:
                    nc.sync.dma_start(out=w1dst, in_=w1src)

            # ---------------- DVE-side constant prep ----------------
            # Ordering matters: everything block 0 needs (band buffer 0,
            # W23T half 0, band13 half 0) is produced first so stage 1 can
            # start ~5us in; the rest fills DVE slack before its consumers.
            BW = 260 * NU + 4  # 2084: scatter view needs [pl4, pl4+2080)
            bufs = [
                pp.tile([128, BW], F32R, tag=f"band{i}", name=f"band{i}")
                for i in range(2)
            ]
            zsrc = pp.tile([128, 256], F32, tag="zsrc")
            nc.vector.memset(zsrc, 0.0)
            nc.vector.tensor_copy(
                bufs[0][:, 0:2048].rearrange("q (g c) -> q g c", c=256),
                zsrc[:, :].rearrange("q (g c) -> q g c", g=1).broadcast_to(
                    [128, 8, 256]
                ),
            )

            ident = pp.tile([128, 128], F32, tag="ident")
            make_identity(nc, ident)

            # W23 = W3@W2 per position on the DVE, W23T via PE transpose,
            # band13 (narrow W23^T block-diag) scatter -- per 128-pos half.
            MULT = mybir.AluOpType.mult
            AX_X = mybir.AxisListType.X
            W23p = pp.tile([128, 2 * O3 * O1], F32, tag="W23p")  # [128, 192]
            prod = pp.tile([128, O3 * O1 * O2], F32, tag="prod")  # [128, 768]
            W23T = pp.tile([32, NBLK * O3 * POS + 12], F32R, tag="W23T")
            band13 = pp.tile([128, 64 * 12 + 12], F32R, tag="band13")  # [128, 780]
            for h in range(2):
                prodv = prod[:, :].rearrange(
                    "q (x o1 o2) -> q x o1 o2", x=O3, o1=O1
                )
                nc.vector.tensor_tensor(
                    prodv,
                    w3p[:, 24 * h : 24 * (h + 1)]
                    .rearrange("q (x g o2) -> q x g o2", x=O3, g=1)
                    .broadcast_to([128, O3, O1, O2]),
                    W2p[:, 256 * h : 256 * (h + 1)]
                    .rearrange("q (g o2 o1) -> q g o1 o2", g=1, o2=O2)
                    .broadcast_to([128, O3, O1, O2]),
                    MULT,
                )
                nc.vector.tensor_reduce(
                    W23p[:, 96 * h : 96 * (h + 1)].rearrange(
                        "q (x o1) -> q x o1", x=O3
                    ),
                    prodv,
                    AX_X,
                    ADD,
                )
                # W23T[o1, 3p + x] = W23[p, x, o1] for this half
                psW = pstp.tile([128, 256], F32, tag="pst", name=f"psW{h}")
                nc.tensor.transpose(
                    psW[0:96, 0:128], W23p[:, 96 * h : 96 * (h + 1)],
                    ident[0:128, 0:128],
                )
                for x in range(O3):
                    dstw = (
                        W23T.bitcast(F32)[
                            0:32, 384 * h + x : 384 * h + x + 384
                        ].rearrange("q (c w) -> q c w", w=3)[:, :, 0]
                    )
                    nc.vector.tensor_copy(dstw, psW[32 * x : 32 * (x + 1), 0:128])
                if h == 0:
                    nc.vector.tensor_copy(
                        band13[:, 0:768].rearrange("q (g c) -> q g c", c=256),
                        zsrc[:, :].rearrange("q (g c) -> q g c", g=1).broadcast_to(
                            [128, 3, 256]
                        ),
                    )
                # band13[32 pl4 + o1, 12 g + 3 pl4 + x] = W23T[o1, 3(4g+pl4)+x]
                for pl4 in range(4):
                    dst13 = (
                        band13[
                            32 * pl4 : 32 * (pl4 + 1),
                            384 * h + 3 * pl4 : 384 * h + 3 * pl4 + 384,
                        ].rearrange("q (g w) -> q g w", w=12)[:, :, 0:O3]
                    )
                    src13 = (
                        W23T.bitcast(F32)[
                            :, 384 * h + 3 * pl4 : 384 * h + 3 * pl4 + 384
                        ].rearrange("q (g w) -> q g w", w=12)[:, :, 0:O3]
                    )
                    nc.vector.tensor_copy(dst13, src13)

            # glf [32, 512] -> glfT chunks: glfT[:, 32k:+32] = glf[:, 128k:+128].T
            glfT = pp.tile([128, 128], F32R, tag="glfT")
            for k in range(4):
                pt = pstp.tile([128, 256], F32, tag="pst", name=f"ptg{k}")
                nc.tensor.transpose(
                    pt[0:128, 0:B], glf_sb[:, 128 * k : 128 * (k + 1)],
                    ident[0:B, 0:B],
                )
                nc.vector.tensor_copy(glfT[:, 32 * k : 32 * k + 32], pt[0:128, 0:B])

            ones_sb = pp.tile([1, B], F32R, tag="ones")
            ones_f32 = pp.tile([1, B], F32, tag="ones32")
            nc.vector.memset(ones_f32, 1.0)
            nc.vector.tensor_copy(ones_sb, ones_f32)

            # b1_sb[q, g] = b1_flat[128 g + q] = b1[4g + (q//32), q%32]
            b1_sb = pp.tile([128, 64 + 1], F32R, tag="b1")
            nc.vector.memset(b1_sb[:, 64:65].bitcast(F32), 0.0)
            ptb = pstp.tile([128, 256], F32, tag="pst", name="ptb")
            nc.tensor.transpose(
                ptb[0:64, 0:128], b1nat[:, 0:64], ident[0:128, 0:128]
            )
            for h in range(2):
                for j in range(4):
                    nc.vector.tensor_copy(
                        b1_sb[32 * j : 32 * (j + 1), 32 * h : 32 * (h + 1)],
                        ptb[32 * h : 32 * (h + 1), 0:128].rearrange(
                            "q (g f) -> q g f", f=4
                        )[:, :, j],
                    )

            # w3b2p[q, (h, x)] = sum_o2 W3[128h+q, x, o2] * b2[128h+q, o2]
            w3b2p = pp.tile([128, 2 * O3], F32, tag="w3b2p")  # [128, 6]
            prod2 = pp.tile([128, O3 * O2], F32, tag="prod2")  # [128, 24]
            for h in range(2):
                prod2v = prod2[:, :].rearrange("q (x o2) -> q x o2", x=O3)
                nc.vector.tensor_tensor(
                    prod2v,
                    w3p[:, 24 * h : 24 * (h + 1)].rearrange(
                        "q (x o2) -> q x o2", x=O3
                    ),
                    b2p[:, 8 * h : 8 * (h + 1)]
                    .rearrange("q (g o2) -> q g o2", g=1)
                    .broadcast_to([128, O3, O2]),
                    MULT,
                )
                nc.vector.tensor_reduce(
                    w3b2p[:, 3 * h : 3 * (h + 1)], prod2v, AX_X, ADD
                )

            # w3b2row[0, 96*blk + 32*x + pl] = (W3@b2)[32*blk + pl, x]
            w3b2row = pp.tile([1, NBLK * O3 * POS], F32, tag="w3b2")  # [1, 768]
            for h in range(2):
                for x in range(O3):
                    j = 3 * h + x
                    psj = pstp.tile([128, 256], F32, tag="pst", name=f"psj{j}")
                    nc.tensor.transpose(
                        psj[0:1, 0:128], w3b2p[:, j : j + 1], ident[0:128, 0:128]
                    )
                    dstj = w3b2row[0:1, :].rearrange(
                        "q (blk x pl) -> q blk x pl", x=O3, pl=POS
                    )[:, 4 * h : 4 * (h + 1), x, :]
                    srcj = psj[0:1, 0:128].rearrange(
                        "q (blk pl) -> q blk pl", pl=POS
                    )
                    nc.scalar.copy(dstj, srcj)

            # basebias = w3b2row + b3 (remapped flat (p,x) -> (blk,x,pl))
            basebias = pp.tile([1, NBLK * O3 * POS], F32, tag="bb")  # [1, 768]
            nc.vector.tensor_tensor(
                basebias[0:1, :].rearrange(
                    "q (blk x pl) -> q blk x pl", x=O3, pl=POS
                ),
                w3b2row[0:1, :].rearrange(
                    "q (blk x pl) -> q blk x pl", x=O3, pl=POS
                ),
                b3nat[0:1, :].rearrange(
                    "q (blk pl x) -> q blk x pl", pl=POS, x=O3
                ),
                ADD,
            )

            beffrow = pp.tile([1, NBLK * O3 * POS], F32R, tag="beff")  # [1, 768]
            out_sb = pp.tile([B, O3 * PP], F32, tag="outsb")  # [32, 768]

            nc.vector.tensor_copy(
                bufs[1][:, 0:2048].rearrange("q (g c) -> q g c", c=256),
                zsrc[:, :].rearrange("q (g c) -> q g c", g=1).broadcast_to(
                    [128, 8, 256]
                ),
            )

            # ---------------- main loop over position blocks ----------------
            # blocks 0-6: 32 positions on the two rotating big bands; the
            # final 32 positions run as two 16-position blocks on dedicated
            # small bands so almost no compute trails the last W1 byte.
            # Stage 2 of block b is emitted after stage 1 of block b+1
            # (software pipelining) so its matmuls never stall the PE on
            # the PSUM->SBUF drains.
            bufs16 = [
                pp.tile([128, 260 * 4 + 4], F32R, tag=f"b16_{i}", name=f"b16_{i}")
                for i in range(2)
            ]

            blocks = [(b, 32, bufs[b % 2]) for b in range(7)]
            blocks += [(7, 16, bufs16[0]), (8, 16, bufs16[1])]
            state = {}

            def emit_scatter(blk, POSb, buf):
                p0 = 32 * blk if POSb == 32 else 224 + 16 * (blk - 7)
                NUb = POSb // 4
                nb = 3 * POSb
                g0 = p0 // 4
                # scatter W23^T blocks into the sliding band
                for pl4 in range(4):
                    dst = (
                        buf[
                            32 * pl4 : 32 * (pl4 + 1),
                            pl4 : pl4 + 260 * NUb,
                        ].rearrange("q (u w) -> q u w", w=260)[:, :, 0:nb]
                        .rearrange("q u (x s) -> q u x s", s=POSb)[:, :, :, 0]
                    )
                    srcs = (
                        W23T.bitcast(F32)[
                            :, 3 * p0 + 3 * pl4 : 3 * p0 + 3 * pl4 + 12 * NUb
                        ].rearrange("q (u w) -> q u w", w=12)[:, :, 0:O3]
                    )
                    nc.vector.tensor_copy(dst, srcs)

            def emit_front(blk, POSb, buf):
                p0 = 32 * blk if POSb == 32 else 224 + 16 * (blk - 7)
                NUb = POSb // 4
                nb = 3 * POSb
                g0 = p0 // 4
                # stage 1: MeffT chunks, 4 i-chunks x NUb accumulating matmuls
                mts = []
                for c in range(4):
                    pst = pstp.tile(
                        [128, WIN], F32, tag="pst", name=f"pst{blk}_{c}"
                    )
                    for u in range(NUb):
                        w1t = w1tiles[(p0 + 4 * u) // 16]
                        lhsT = w1t[:].rearrange("q (v i) -> q v i", v=4)[
                            :, ((p0 + 4 * u) % 16) // 4, 128 * c : 128 * (c + 1)
                        ]
                        nc.tensor.matmul(
                            pst[:, 0:nb] if POSb == 16 else pst,
                            lhsT=lhsT,
                            rhs=buf[:, 256 * u : 256 * u + nb]
                            if POSb == 16
                            else buf[:, 256 * u : 256 * (u + 1)],
                            start=(u == 0),
                            stop=(u == NUb - 1),
                        )
                    mt = mtp.tile([128, WIN], F32R, tag="mt", name=f"mt{blk}_{c}")
                    mts.append(mt)
                    if c % 2 == 0:
                        nc.scalar.copy(mt[:, 0:nb], pst[:, 0:nb])
                    else:
                        nc.vector.tensor_copy(mt[:, 0:nb], pst[:, 0:nb])

                # beff part 1: W23@b1 via narrow band (NUb tiny matmuls)
                psb = psbp.tile([1, 96], F32, tag="psb", name=f"psb{blk}")
                for u in range(NUb):
                    g = g0 + u
                    nc.tensor.matmul(
                        psb[0:1, 12 * u : 12 * (u + 1)],
                        lhsT=b1_sb[:, g : g + 1],
                        rhs=band13[:, 12 * g : 12 * (g + 1)],
                        start=(u == 0),
                        stop=(u == NUb - 1),
                    )
                state[blk] = (POSb, p0, NUb, nb, psb, mts)

            def emit_tail(blk):
                POSb, p0, NUb, nb, psb, mts = state.pop(blk)
                half = 0 if POSb == 32 else blk - 7
                # beffrow_blk = remap(psb) + basebias_blk  (to (x, pl) layout)
                dstb = beffrow[0:1, 3 * p0 : 3 * p0 + nb].rearrange(
                    "q (x u pl4) -> q u pl4 x", x=O3, u=NUb
                )
                srcb = psb[0:1, 0 : 12 * NUb].rearrange(
                    "q (u pl4 x) -> q u pl4 x", u=NUb, x=O3
                )
                if POSb == 32:
                    srcc = basebias[0:1, 96 * blk : 96 * (blk + 1)].rearrange(
                        "q (x u pl4) -> q u pl4 x", x=O3, u=NUb
                    )
                else:
                    srcc = basebias[0:1, 672:768].rearrange(
                        "q (x hh u pl4) -> q hh u pl4 x", x=O3, hh=2, u=4
                    )[:, half]
                nc.vector.tensor_tensor(dstb, srcb, srcc, ADD)

                # stage 2: py[b, (x, pl)] = glf @ MeffT + beff
                py = psyp.tile([B, WIN], F32, tag="py", name=f"py{blk}")
                for c in range(3):
                    nc.tensor.matmul(
                        py[:, 0:nb],
                        lhsT=glfT[:, 32 * c : 32 * (c + 1)],
                        rhs=mts[c][:, 0:nb],
                        start=(c == 0),
                        stop=False,
                    )
                nc.tensor.matmul(
                    py[:, 0:nb],
                    lhsT=ones_sb,
                    rhs=beffrow[0:1, 3 * p0 : 3 * p0 + nb],
                    start=False,
                    stop=False,
                )
                nc.tensor.matmul(
                    py[:, 0:nb],
                    lhsT=glfT[:, 96:128],
                    rhs=mts[3][:, 0:nb],
                    start=False,
                    stop=True,
                )
                # drain py -> out_sb (global (x, p) layout)
                dsto = out_sb[:, :].rearrange("q (x p) -> q x p", x=O3)[
                    :, :, p0 : p0 + POSb
                ]
                nc.vector.tensor_copy(
                    dsto, py[:, 0:nb].rearrange("q (x pl) -> q x pl", x=O3)
                )
                if blk == 7:
                    # ship blocks 0..7a while the last block finishes
                    nc.scalar.dma_start(
                        out=out[:, :, 0:240],
                        in_=out_sb[:, :].rearrange("q (x p) -> q x p", x=O3)[
                            :, :, 0:240
                        ],
                    )
                elif blk == 8:
                    # last block on the idle sync queue -- shortest tail
                    nc.sync.dma_start(
                        out=out[:, :, 240:256],
                        in_=out_sb[:, :].rearrange("q (x p) -> q x p", x=O3)[
                            :, :, 240:256
                        ],
                    )

            emit_scatter(*blocks[0])
            for i, (blk, POSb, buf) in enumerate(blocks):
                if blk == 3:
                    # zero the small bands mid-stream: the DVE has slack
                    # here, and they are needed only from block 7 on
                    for bi in range(2):
                        nc.vector.tensor_copy(
                            bufs16[bi][:, 0:1024].rearrange(
                                "q (g c) -> q g c", c=256
                            ),
                            zsrc[:, :].rearrange(
                                "q (g c) -> q g c", g=1
                            ).broadcast_to([128, 4, 256]),
                        )
                if i + 1 < len(blocks):
                    # scatter for the NEXT block goes first in the DVE queue
                    # so this block's drains/py never delay it
                    emit_scatter(*blocks[i + 1])
                emit_front(blk, POSb, buf)
                if i > 0 and blocks[i - 1][0] in state:
                    emit_tail(blocks[i - 1][0])
                if POSb == 16:
                    emit_tail(blk)

    nc.compile()
    return nc


def _get_nc():
    if "nc" not in _CACHE:
        _CACHE["nc"] = _build_nc()
    return _CACHE["nc"]


def _make_in_maps(inputs):
    glf = np.ascontiguousarray(
        np.asarray(inputs["glf"], dtype=np.float32).reshape(B, I)
    )
    ins = {k: np.asarray(inputs[k], dtype=np.float32) for k in
           ("W1", "b1", "W2", "b2", "W3", "b3")}
    in_maps = []
    for c in range(NCORES):
        sl = slice(c * PP, (c + 1) * PP)
        in_maps.append(
            {
                "W1": np.ascontiguousarray(ins["W1"][sl]),
                "b1": np.ascontiguousarray(ins["b1"][sl]),
                "W2": np.ascontiguousarray(ins["W2"][sl]),
                "b2": np.ascontiguousarray(ins["b2"][sl]),
                "W3": np.ascontiguousarray(ins["W3"][sl]),
                "b3": np.ascontiguousarray(ins["b3"][sl]),
                "glf": glf,
            }
        )
    return in_maps


def run(inputs, trace=False):
    """Run on the 8 NeuronCores; returns (out_full, BassKernelResults)."""
    from concourse.bass_utils import run_bass_kernel_spmd

    nc = _get_nc()
    res = run_bass_kernel_spmd(
        nc, _make_in_maps(inputs), list(range(NCORES)), trace=trace
    )
    out_full = np.empty((B, O3, P_FULL), dtype=np.float32)
    for c in range(NCORES):
        out_full[:, :, c * PP : (c + 1) * PP] = res.results[c]["out"]
    return out_full, res


def kernel(**inputs):
    out, _ = run(inputs, trace=False)
    return out


# revision 52
# speedup vs baseline: 1.0016x; 1.0016x over previous
"""Trainium2 Bass kernel for per-position FC decoder stack.

out[b, o3, p] = W3[p] @ (W2[p] @ (W1[p] @ glf[b] + b1[p]) + b2[p]) + b3[p]

All layers are linear, so we fold the ENTIRE stack into a per-position affine
map Meff[p] = W3[p]@W2[p]@W1[p] ([3, 512]) and beff[p] = W3@(W2@b1+b2)+b3
([3]).  MeffT is computed directly (no W1 transpose) by using W1 natural-
layout 128-row chunks as the stationary operand against a zero-padded
block-diagonal W23^T band (moving, 256-wide windows so float32r runs at full
PE rate).  Stage 2 applies MeffT to glf^T (4 accumulating f32r matmuls + one
bias-row matmul) producing the output block [B, (o3, p)] straight in PSUM.

W23 = W3@W2 is itself computed on the PE with a small block-diag W3^T band.
beff rows come from tiny accumulating matmuls against a narrow W23^T band.

Sharding: positions (2048) split across 8 cores; glf replicated.  W1 per core
(16 MiB) is the DMA roofline; all W1 tiles stay resident in SBUF so the DMA
stream never stalls on compute, and per-block output DMAs keep the tail short.
"""

import sys

if "/opt/trn_rl_repo" not in sys.path:
    sys.path.insert(0, "/opt/trn_rl_repo")

import numpy as np

# Problem constants (hardcoded per contest contract)
P_FULL = 2048
NCORES = 8
PP = P_FULL // NCORES  # 256 positions per core
B = 32
I = 512
O1 = 32
O2 = 8
O3 = 3
NT = 16    # W1 tiles of [128, 4*512] (16 positions each)
NBLK = 8   # blocks of 32 positions
POS = 32   # positions per block
NU = 8     # u-groups (4 positions) per block
WIN = 256  # band window width per u-group

_CACHE = {}


def _build_nc():
    import concourse.bass as bass
    import concourse.mybir as mybir
    import concourse.tile as tile
    from concourse import bacc
    from concourse.masks import make_identity

    F32 = mybir.dt.float32
    F32R = mybir.dt.float32r
    ADD = mybir.AluOpType.add

    nc = bacc.Bacc(
        "TRN2", target_bir_lowering=False, debug=False, num_devices=NCORES
    )
    W1 = nc.declare_dram_parameter("W1", [PP, O1, I], F32R, isOutput=False)
    b1 = nc.declare_dram_parameter("b1", [PP, O1], F32R, isOutput=False)
    W2 = nc.declare_dram_parameter("W2", [PP, O2, O1], F32, isOutput=False)
    b2 = nc.declare_dram_parameter("b2", [PP, O2], F32R, isOutput=False)
    W3 = nc.declare_dram_parameter("W3", [PP, O3, O2], F32, isOutput=False)
    b3 = nc.declare_dram_parameter("b3", [PP, O3], F32, isOutput=False)
    glf = nc.declare_dram_parameter("glf", [B, I], F32, isOutput=False)
    out = nc.declare_dram_parameter("out", [B, O3, PP], F32, isOutput=True)

    with tile.TileContext(nc) as tc:
        with (
            tc.tile_pool(name="persist", bufs=1) as pp,
            tc.tile_pool(name="w1s", bufs=NT) as w1p,
            tc.tile_pool(name="mt", bufs=8) as mtp,
            tc.tile_pool(name="pst", bufs=5, space="PSUM") as pstp,
            tc.tile_pool(name="psy", bufs=1, space="PSUM") as psyp,
            tc.tile_pool(name="psb", bufs=2, space="PSUM") as psbp,
        ):
            # ---------------- prep DMAs ----------------
            # scalar queue: small inputs (positions on partitions where used
            # by the DVE-side W23 = W3@W2 computation)
            # w3p[q, h, (x, o2)] = W3[128h + q, x, o2]
            w3p = pp.tile([128, 2 * O3 * O2], F32, tag="w3p")  # [128, 48]
            nc.scalar.dma_start(
                out=w3p[:].rearrange("q (h c) -> q h c", h=2),
                in_=W3[:].rearrange("p x o -> p (x o)").rearrange(
                    "(h q) c -> q h c", q=128
                ),
            )
            # W2p[q, h, (o2, o1)] = W2[128h + q, o2, o1]
            W2p = pp.tile([128, 2 * O2 * O1], F32, tag="W2p")  # [128, 512]
            nc.sync.dma_start(
                out=W2p[:].rearrange("q (h c) -> q h c", h=2),
                in_=W2[:].rearrange("p o i -> p (o i)").rearrange(
                    "(h q) c -> q h c", q=128
                ),
            )
            # b2p[q, h, o2] = b2[128h + q, o2]
            b2p = pp.tile([128, 2 * O2], F32, tag="b2p")  # [128, 16]
            nc.scalar.dma_start(
                out=b2p[:].rearrange("q (h o) -> q h o", h=2),
                in_=b2[:].bitcast(F32).rearrange("(h q) o -> q h o", q=128),
            )
            # b1B[q, c] = b1_flat[128 q + c]: 512B elements (full DMA rate);
            # its PE transpose is exactly the b1_sb layout
            b1B = pp.tile([64, 128], F32, tag="b1B")
            nc.scalar.dma_start(
                out=b1B,
                in_=b1[:].bitcast(F32).rearrange("p o -> (p o)").rearrange(
                    "(q c) -> q c", c=128
                ),
            )
            glf_sb = pp.tile([B, I], F32, tag="glf")
            nc.scalar.dma_start(out=glf_sb, in_=glf[:])
            # b3 flat; remapped to block layout inside the basebias add
            b3nat = pp.tile([1, PP * O3], F32, tag="b3nat")  # [1, 768]
            nc.scalar.dma_start(
                out=b3nat,
                in_=b3[:].rearrange("p x -> (p x)").rearrange("(o f) -> o f", o=1),
            )

            # ---------------- W1 stream (all tiles resident) ----------------
            w1tiles = {}
            for t in range(NT):
                w1t = w1p.tile([128, 4 * I], F32R, tag="w1t", name=f"w1t{t}")
                w1tiles[t] = w1t
                w1src = (
                    W1[:]
                    .rearrange("p o i -> (p o) i")[512 * t : 512 * (t + 1), :]
                    .rearrange("(u q) i -> q u i", q=128)
                )
                w1dst = w1t[:].rearrange("q (u i) -> q u i", u=4)
                if t >= NT - 2:
                    # split the last tiles by i-chunk so the final c-chains
                    # can start before the full tile lands
                    for ic in range(4):
                        nc.sync.dma_start(
                            out=w1dst[:, :, 128 * ic : 128 * (ic + 1)],
                            in_=w1src[:, :, 128 * ic : 128 * (ic + 1)],
                        )
                else:
                    nc.sync.dma_start(out=w1dst, in_=w1src)

            # ---------------- DVE-side constant prep ----------------
            # Ordering matters: everything block 0 needs (band buffer 0,
            # W23T half 0, band13 half 0) is produced first so stage 1 can
            # start ~5us in; the rest fills DVE slack before its consumers.
            BW = 260 * NU + 4  # 2084: scatter view needs [pl4, pl4+2080)
            bufs = [
                pp.tile([128, BW], F32R, tag=f"band{i}", name=f"band{i}")
                for i in range(2)
            ]
            zsrc = pp.tile([128, 256], F32, tag="zsrc")
            nc.vector.memset(zsrc, 0.0)
            nc.vector.tensor_copy(
                bufs[0][:, 0:2048].rearrange("q (g c) -> q g c", c=256),
                zsrc[:, :].rearrange("q (g c) -> q g c", g=1).broadcast_to(
                    [128, 8, 256]
                ),
            )

            ident = pp.tile([128, 128], F32, tag="ident")
            make_identity(nc, ident)

            # W23 = W3@W2 per position on the DVE, W23T via PE transpose,
            # band13 (narrow W23^T block-diag) scatter -- per 128-pos half.
            MULT = mybir.AluOpType.mult
            AX_X = mybir.AxisListType.X
            W23p = pp.tile([128, 2 * O3 * O1], F32, tag="W23p")  # [128, 192]
            prod = pp.tile([128, O3 * O1 * O2], F32, tag="prod")  # [128, 768]
            W23T = pp.tile([32, NBLK * O3 * POS + 12], F32R, tag="W23T")
            band13 = pp.tile([128, 64 * 12 + 12], F32R, tag="band13")  # [128, 780]
            for h in range(2):
                prodv = prod[:, :].rearrange(
                    "q (x o1 o2) -> q x o1 o2", x=O3, o1=O1
                )
                nc.vector.tensor_tensor(
                    prodv,
                    w3p[:, 24 * h : 24 * (h + 1)]
                    .rearrange("q (x g o2) -> q x g o2", x=O3, g=1)
                    .broadcast_to([128, O3, O1, O2]),
                    W2p[:, 256 * h : 256 * (h + 1)]
                    .rearrange("q (g o2 o1) -> q g o1 o2", g=1, o2=O2)
                    .broadcast_to([128, O3, O1, O2]),
                    MULT,
                )
                nc.vector.tensor_reduce(
                    W23p[:, 96 * h : 96 * (h + 1)].rearrange(
                        "q (x o1) -> q x o1", x=O3
                    ),
                    prodv,
                    AX_X,
                    ADD,
                )
                # W23T[o1, 3p + x] = W23[p, x, o1] for this half
                psW = pstp.tile([128, 256], F32, tag="pst", name=f"psW{h}")
                nc.tensor.transpose(
                    psW[0:96, 0:128], W23p[:, 96 * h : 96 * (h + 1)],
                    ident[0:128, 0:128],
                )
                for x in range(O3):
                    dstw = (
                        W23T.bitcast(F32)[
                            0:32, 384 * h + x : 384 * h + x + 384
                        ].rearrange("q (c w) -> q c w", w=3)[:, :, 0]
                    )
                    nc.vector.tensor_copy(dstw, psW[32 * x : 32 * (x + 1), 0:128])
                if h == 0:
                    nc.vector.tensor_copy(
                        band13[:, 0:768].rearrange("q (g c) -> q g c", c=256),
                        zsrc[:, :].rearrange("q (g c) -> q g c", g=1).broadcast_to(
                            [128, 3, 256]
                        ),
                    )
                # band13[32 pl4 + o1, 12 g + 3 pl4 + x] = W23T[o1, 3(4g+pl4)+x]
                for pl4 in range(4):
                    dst13 = (
                        band13[
                            32 * pl4 : 32 * (pl4 + 1),
                            384 * h + 3 * pl4 : 384 * h + 3 * pl4 + 384,
                        ].rearrange("q (g w) -> q g w", w=12)[:, :, 0:O3]
                    )
                    src13 = (
                        W23T.bitcast(F32)[
                            :, 384 * h + 3 * pl4 : 384 * h + 3 * pl4 + 384
                        ].rearrange("q (g w) -> q g w", w=12)[:, :, 0:O3]
                    )
                    nc.vector.tensor_copy(dst13, src13)

            # glf [32, 512] -> glfT chunks: glfT[:, 32k:+32] = glf[:, 128k:+128].T
            glfT = pp.tile([128, 128], F32R, tag="glfT")
            for k in range(4):
                pt = pstp.tile([128, 256], F32, tag="pst", name=f"ptg{k}")
                nc.tensor.transpose(
                    pt[0:128, 0:B], glf_sb[:, 128 * k : 128 * (k + 1)],
                    ident[0:B, 0:B],
                )
                nc.vector.tensor_copy(glfT[:, 32 * k : 32 * k + 32], pt[0:128, 0:B])

            ones_sb = pp.tile([1, B], F32R, tag="ones")
            ones_f32 = pp.tile([1, B], F32, tag="ones32")
            nc.vector.memset(ones_f32, 1.0)
            nc.vector.tensor_copy(ones_sb, ones_f32)

            # b1_sb[q, g] = b1_flat[128 g + q] = transpose(b1B)
            b1_sb = pp.tile([128, 64 + 1], F32R, tag="b1")
            nc.vector.memset(b1_sb[:, 64:65].bitcast(F32), 0.0)
            ptb = pstp.tile([128, 256], F32, tag="pst", name="ptb")
            nc.tensor.transpose(
                ptb[0:128, 0:64], b1B[0:64, 0:128], ident[0:64, 0:64]
            )
            nc.vector.tensor_copy(b1_sb[:, 0:64], ptb[0:128, 0:64])

            # w3b2p[q, (h, x)] = sum_o2 W3[128h+q, x, o2] * b2[128h+q, o2]
            w3b2p = pp.tile([128, 2 * O3], F32, tag="w3b2p")  # [128, 6]
            prod2 = pp.tile([128, O3 * O2], F32, tag="prod2")  # [128, 24]
            for h in range(2):
                prod2v = prod2[:, :].rearrange("q (x o2) -> q x o2", x=O3)
                nc.vector.tensor_tensor(
                    prod2v,
                    w3p[:, 24 * h : 24 * (h + 1)].rearrange(
                        "q (x o2) -> q x o2", x=O3
                    ),
                    b2p[:, 8 * h : 8 * (h + 1)]
                    .rearrange("q (g o2) -> q g o2", g=1)
                    .broadcast_to([128, O3, O2]),
                    MULT,
                )
                nc.vector.tensor_reduce(
                    w3b2p[:, 3 * h : 3 * (h + 1)], prod2v, AX_X, ADD
                )

            # w3b2row[0, 96*blk + 32*x + pl] = (W3@b2)[32*blk + pl, x]
            w3b2row = pp.tile([1, NBLK * O3 * POS], F32, tag="w3b2")  # [1, 768]
            for h in range(2):
                for x in range(O3):
                    j = 3 * h + x
                    psj = pstp.tile([128, 256], F32, tag="pst", name=f"psj{j}")
                    nc.tensor.transpose(
                        psj[0:1, 0:128], w3b2p[:, j : j + 1], ident[0:128, 0:128]
                    )
                    dstj = w3b2row[0:1, :].rearrange(
                        "q (blk x pl) -> q blk x pl", x=O3, pl=POS
                    )[:, 4 * h : 4 * (h + 1), x, :]
                    srcj = psj[0:1, 0:128].rearrange(
                        "q (blk pl) -> q blk pl", pl=POS
                    )
                    nc.scalar.copy(dstj, srcj)

            # basebias = w3b2row + b3 (remapped flat (p,x) -> (blk,x,pl))
            basebias = pp.tile([1, NBLK * O3 * POS], F32, tag="bb")  # [1, 768]
            nc.vector.tensor_tensor(
                basebias[0:1, :].rearrange(
                    "q (blk x pl) -> q blk x pl", x=O3, pl=POS
                ),
                w3b2row[0:1, :].rearrange(
                    "q (blk x pl) -> q blk x pl", x=O3, pl=POS
                ),
                b3nat[0:1, :].rearrange(
                    "q (blk pl x) -> q blk x pl", pl=POS, x=O3
                ),
                ADD,
            )

            beffrow = pp.tile([1, NBLK * O3 * POS], F32R, tag="beff")  # [1, 768]
            out_sb = pp.tile([B, O3 * PP], F32, tag="outsb")  # [32, 768]

            nc.vector.tensor_copy(
                bufs[1][:, 0:2048].rearrange("q (g c) -> q g c", c=256),
                zsrc[:, :].rearrange("q (g c) -> q g c", g=1).broadcast_to(
                    [128, 8, 256]
                ),
            )

            # ---------------- main loop over position blocks ----------------
            # blocks 0-6: 32 positions on the two rotating big bands; the
            # final 32 positions run as two 16-position blocks on dedicated
            # small bands so almost no compute trails the last W1 byte.
            # Stage 2 of block b is emitted after stage 1 of block b+1
            # (software pipelining) so its matmuls never stall the PE on
            # the PSUM->SBUF drains.
            bufs16 = [
                pp.tile([128, 260 * 4 + 4], F32R, tag=f"b16_{i}", name=f"b16_{i}")
                for i in range(2)
            ]

            blocks = [(b, 32, bufs[b % 2]) for b in range(7)]
            blocks += [(7, 16, bufs16[0]), (8, 16, bufs16[1])]
            state = {}

            def emit_scatter(blk, POSb, buf):
                p0 = 32 * blk if POSb == 32 else 224 + 16 * (blk - 7)
                NUb = POSb // 4
                nb = 3 * POSb
                g0 = p0 // 4
                # scatter W23^T blocks into the sliding band
                for pl4 in range(4):
                    dst = (
                        buf[
                            32 * pl4 : 32 * (pl4 + 1),
                            pl4 : pl4 + 260 * NUb,
                        ].rearrange("q (u w) -> q u w", w=260)[:, :, 0:nb]
                        .rearrange("q u (x s) -> q u x s", s=POSb)[:, :, :, 0]
                    )
                    srcs = (
                        W23T.bitcast(F32)[
                            :, 3 * p0 + 3 * pl4 : 3 * p0 + 3 * pl4 + 12 * NUb
                        ].rearrange("q (u w) -> q u w", w=12)[:, :, 0:O3]
                    )
                    nc.vector.tensor_copy(dst, srcs)

            def emit_front(blk, POSb, buf):
                p0 = 32 * blk if POSb == 32 else 224 + 16 * (blk - 7)
                NUb = POSb // 4
                nb = 3 * POSb
                g0 = p0 // 4
                # stage 1: MeffT chunks, 4 i-chunks x NUb accumulating matmuls
                mts = []
                for c in range(4):
                    pst = pstp.tile(
                        [128, WIN], F32, tag="pst", name=f"pst{blk}_{c}"
                    )
                    for u in range(NUb):
                        w1t = w1tiles[(p0 + 4 * u) // 16]
                        lhsT = w1t[:].rearrange("q (v i) -> q v i", v=4)[
                            :, ((p0 + 4 * u) % 16) // 4, 128 * c : 128 * (c + 1)
                        ]
                        nc.tensor.matmul(
                            pst[:, 0:nb] if POSb == 16 else pst,
                            lhsT=lhsT,
                            rhs=buf[:, 256 * u : 256 * u + nb]
                            if POSb == 16
                            else buf[:, 256 * u : 256 * (u + 1)],
                            start=(u == 0),
                            stop=(u == NUb - 1),
                        )
                    mt = mtp.tile([128, WIN], F32R, tag="mt", name=f"mt{blk}_{c}")
                    mts.append(mt)
                    if c % 2 == 0:
                        nc.scalar.copy(mt[:, 0:nb], pst[:, 0:nb])
                    else:
                        nc.vector.tensor_copy(mt[:, 0:nb], pst[:, 0:nb])

                # beff part 1: W23@b1 via narrow band (NUb tiny matmuls)
                psb = psbp.tile([1, 96], F32, tag="psb", name=f"psb{blk}")
                for u in range(NUb):
                    g = g0 + u
                    nc.tensor.matmul(
                        psb[0:1, 12 * u : 12 * (u + 1)],
                        lhsT=b1_sb[:, g : g + 1],
                        rhs=band13[:, 12 * g : 12 * (g + 1)],
                        start=(u == 0),
                        stop=(u == NUb - 1),
                    )
                state[blk] = (POSb, p0, NUb, nb, psb, mts)

            def emit_tail(blk):
                POSb, p0, NUb, nb, psb, mts = state.pop(blk)
                half = 0 if POSb == 32 else blk - 7
                # beffrow_blk = remap(psb) + basebias_blk  (to (x, pl) layout)
                dstb = beffrow[0:1, 3 * p0 : 3 * p0 + nb].rearrange(
                    "q (x u pl4) -> q u pl4 x", x=O3, u=NUb
                )
                srcb = psb[0:1, 0 : 12 * NUb].rearrange(
                    "q (u pl4 x) -> q u pl4 x", u=NUb, x=O3
                )
                if POSb == 32:
                    srcc = basebias[0:1, 96 * blk : 96 * (blk + 1)].rearrange(
                        "q (x u pl4) -> q u pl4 x", x=O3, u=NUb
                    )
                else:
                    srcc = basebias[0:1, 672:768].rearrange(
                        "q (x hh u pl4) -> q hh u pl4 x", x=O3, hh=2, u=4
                    )[:, half]
                nc.vector.tensor_tensor(dstb, srcb, srcc, ADD)

                # stage 2: py[b, (x, pl)] = glf @ MeffT + beff
                py = psyp.tile([B, WIN], F32, tag="py", name=f"py{blk}")
                for c in range(3):
                    nc.tensor.matmul(
                        py[:, 0:nb],
                        lhsT=glfT[:, 32 * c : 32 * (c + 1)],
                        rhs=mts[c][:, 0:nb],
                        start=(c == 0),
                        stop=False,
                    )
                nc.tensor.matmul(
                    py[:, 0:nb],
                    lhsT=ones_sb,
                    rhs=beffrow[0:1, 3 * p0 : 3 * p0 + nb],
                    start=False,
                    stop=False,
                )
                nc.tensor.matmul(
                    py[:, 0:nb],
                    lhsT=glfT[:, 96:128],
                    rhs=mts[3][:, 0:nb],
                    start=False,
                    stop=True,
                )
                # drain py -> out_sb (global (x, p) layout)
                dsto = out_sb[:, :].rearrange("q (x p) -> q x p", x=O3)[
                    :, :, p0 : p0 + POSb
                ]
                nc.vector.tensor_copy(
                    dsto, py[:, 0:nb].rearrange("q (x pl) -> q x pl", x=O3)
                )
                if blk == 7:
                    # ship blocks 0..7a while the last block finishes
                    nc.scalar.dma_start(
                        out=out[:, :, 0:240],
                        in_=out_sb[:, :].rearrange("q (x p) -> q x p", x=O3)[
                            :, :, 0:240
                        ],
                    )
                elif blk == 8:
                    # last block on the idle sync queue -- shortest tail
                    nc.sync.dma_start(
                        out=out[:, :, 240:256],
                        in_=out_sb[:, :].rearrange("q (x p) -> q x p", x=O3)[
                            :, :, 240:256
                        ],
                    )

            emit_scatter(*blocks[0])
            for i, (blk, POSb, buf) in enumerate(blocks):
                if blk == 3:
                    # zero the small bands mid-stream: the DVE has slack
                    # here, and they are needed only from block 7 on
                    for bi in range(2):
                        nc.vector.tensor_copy(
                            bufs16[bi][:, 0:1024].rearrange(
                                "q (g c) -> q g c", c=256
                            ),
                            zsrc[:, :].rearrange(
                                "q (g c) -> q g c", g=1
                            ).broadcast_to([128, 4, 256]),
                        )
                if i + 1 < len(blocks):
                    # scatter for the NEXT block goes first in the DVE queue
                    # so this block's drains/py never delay it
                    emit_scatter(*blocks[i + 1])
                emit_front(blk, POSb, buf)
                if i > 0 and blocks[i - 1][0] in state:
                    emit_tail(blocks[i - 1][0])
                if POSb == 16:
                    emit_tail(blk)

    nc.compile()
    return nc


def _get_nc():
    if "nc" not in _CACHE:
        _CACHE["nc"] = _build_nc()
    return _CACHE["nc"]


def _make_in_maps(inputs):
    glf = np.ascontiguousarray(
        np.asarray(inputs["glf"], dtype=np.float32).reshape(B, I)
    )
    ins = {k: np.asarray(inputs[k], dtype=np.float32) for k in
           ("W1", "b1", "W2", "b2", "W3", "b3")}
    in_maps = []
    for c in range(NCORES):
        sl = slice(c * PP, (c + 1) * PP)
        in_maps.append(
            {
                "W1": np.ascontiguousarray(ins["W1"][sl]),
                "b1": np.ascontiguousarray(ins["b1"][sl]),
                "W2": np.ascontiguousarray(ins["W2"][sl]),
                "b2": np.ascontiguousarray(ins["b2"][sl]),
                "W3": np.ascontiguousarray(ins["W3"][sl]),
                "b3": np.ascontiguousarray(ins["b3"][sl]),
                "glf": glf,
            }
        )
    return in_maps


def run(inputs, trace=False):
    """Run on the 8 NeuronCores; returns (out_full, BassKernelResults)."""
    from concourse.bass_utils import run_bass_kernel_spmd

    nc = _get_nc()
    res = run_bass_kernel_spmd(
        nc, _make_in_maps(inputs), list(range(NCORES)), trace=trace
    )
    out_full = np.empty((B, O3, P_FULL), dtype=np.float32)
    for c in range(NCORES):
        out_full[:, :, c * PP : (c + 1) * PP] = res.results[c]["out"]
    return out_full, res


def kernel(**inputs):
    out, _ = run(inputs, trace=False)
    return out


# revision 53
# speedup vs baseline: 1.0088x; 1.0071x over previous
"""Trainium2 Bass kernel for per-position FC decoder stack.

out[b, o3, p] = W3[p] @ (W2[p] @ (W1[p] @ glf[b] + b1[p]) + b2[p]) + b3[p]

All layers are linear, so we fold the ENTIRE stack into a per-position affine
map Meff[p] = W3[p]@W2[p]@W1[p] ([3, 512]) and beff[p] = W3@(W2@b1+b2)+b3
([3]).  MeffT is computed directly (no W1 transpose) by using W1 natural-
layout 128-row chunks as the stationary operand against a zero-padded
block-diagonal W23^T band (moving, 256-wide windows so float32r runs at full
PE rate).  Stage 2 applies MeffT to glf^T (4 accumulating f32r matmuls + one
bias-row matmul) producing the output block [B, (o3, p)] straight in PSUM.

W23 = W3@W2 is itself computed on the PE with a small block-diag W3^T band.
beff rows come from tiny accumulating matmuls against a narrow W23^T band.

Sharding: positions (2048) split across 8 cores; glf replicated.  W1 per core
(16 MiB) is the DMA roofline; all W1 tiles stay resident in SBUF so the DMA
stream never stalls on compute, and per-block output DMAs keep the tail short.
"""

import sys

if "/opt/trn_rl_repo" not in sys.path:
    sys.path.insert(0, "/opt/trn_rl_repo")

import numpy as np

# Problem constants (hardcoded per contest contract)
P_FULL = 2048
NCORES = 8
PP = P_FULL // NCORES  # 256 positions per core
B = 32
I = 512
O1 = 32
O2 = 8
O3 = 3
NT = 16    # W1 tiles of [128, 4*512] (16 positions each)
NBLK = 8   # blocks of 32 positions
POS = 32   # positions per block
NU = 8     # u-groups (4 positions) per block
WIN = 256  # band window width per u-group

_CACHE = {}


def _build_nc():
    import concourse.bass as bass
    import concourse.mybir as mybir
    import concourse.tile as tile
    from concourse import bacc
    from concourse.masks import make_identity

    F32 = mybir.dt.float32
    F32R = mybir.dt.float32r
    ADD = mybir.AluOpType.add

    nc = bacc.Bacc(
        "TRN2", target_bir_lowering=False, debug=False, num_devices=NCORES
    )
    W1 = nc.declare_dram_parameter("W1", [PP, O1, I], F32R, isOutput=False)
    b1 = nc.declare_dram_parameter("b1", [PP, O1], F32R, isOutput=False)
    W2 = nc.declare_dram_parameter("W2", [PP, O2, O1], F32, isOutput=False)
    b2 = nc.declare_dram_parameter("b2", [PP, O2], F32R, isOutput=False)
    W3 = nc.declare_dram_parameter("W3", [PP, O3, O2], F32, isOutput=False)
    b3 = nc.declare_dram_parameter("b3", [PP, O3], F32, isOutput=False)
    glf = nc.declare_dram_parameter("glf", [B, I], F32, isOutput=False)
    out = nc.declare_dram_parameter("out", [B, O3, PP], F32, isOutput=True)

    with tile.TileContext(nc) as tc:
        with (
            tc.tile_pool(name="persist", bufs=1) as pp,
            tc.tile_pool(name="w1s", bufs=NT) as w1p,
            tc.tile_pool(name="mt", bufs=8) as mtp,
            tc.tile_pool(name="pst", bufs=5, space="PSUM") as pstp,
            tc.tile_pool(name="psy", bufs=1, space="PSUM") as psyp,
            tc.tile_pool(name="psb", bufs=2, space="PSUM") as psbp,
        ):
            # ---------------- prep DMAs ----------------
            # scalar queue: small inputs (positions on partitions where used
            # by the DVE-side W23 = W3@W2 computation)
            # w3p[q, h, (x, o2)] = W3[128h + q, x, o2]
            w3p = pp.tile([128, 2 * O3 * O2], F32, tag="w3p")  # [128, 48]
            nc.scalar.dma_start(
                out=w3p[:].rearrange("q (h c) -> q h c", h=2),
                in_=W3[:].rearrange("p x o -> p (x o)").rearrange(
                    "(h q) c -> q h c", q=128
                ),
            )
            # b2p[q, h, o2] = b2[128h + q, o2]
            b2p = pp.tile([128, 2 * O2], F32, tag="b2p")  # [128, 16]
            nc.scalar.dma_start(
                out=b2p[:].rearrange("q (h o) -> q h o", h=2),
                in_=b2[:].bitcast(F32).rearrange("(h q) o -> q h o", q=128),
            )
            # b1B[q, c] = b1_flat[128 q + c]: 512B elements (full DMA rate);
            # its PE transpose is exactly the b1_sb layout
            b1B = pp.tile([64, 128], F32, tag="b1B")
            nc.scalar.dma_start(
                out=b1B,
                in_=b1[:].bitcast(F32).rearrange("p o -> (p o)").rearrange(
                    "(q c) -> q c", c=128
                ),
            )
            glf_sb = pp.tile([B, I], F32, tag="glf")
            nc.scalar.dma_start(out=glf_sb, in_=glf[:])
            # b3 flat; remapped to block layout inside the basebias add
            b3nat = pp.tile([1, PP * O3], F32, tag="b3nat")  # [1, 768]
            nc.scalar.dma_start(
                out=b3nat,
                in_=b3[:].rearrange("p x -> (p x)").rearrange("(o f) -> o f", o=1),
            )

            # W2p[q, h, (o2, o1)] = W2[128h + q, o2, o1] (loaded second on
            # the sync queue: w1t0's transfer hides its descriptor-gen)
            W2p = pp.tile([128, 2 * O2 * O1], F32, tag="W2p")  # [128, 512]

            # ---------------- W1 stream (all tiles resident) ----------------
            w1tiles = {}
            for t in range(NT):
                w1t = w1p.tile([128, 4 * I], F32R, tag="w1t", name=f"w1t{t}")
                w1tiles[t] = w1t
                w1src = (
                    W1[:]
                    .rearrange("p o i -> (p o) i")[512 * t : 512 * (t + 1), :]
                    .rearrange("(u q) i -> q u i", q=128)
                )
                w1dst = w1t[:].rearrange("q (u i) -> q u i", u=4)
                if t >= NT - 2:
                    # split the last tiles by i-chunk so the final c-chains
                    # can start before the full tile lands
                    for ic in range(4):
                        nc.sync.dma_start(
                            out=w1dst[:, :, 128 * ic : 128 * (ic + 1)],
                            in_=w1src[:, :, 128 * ic : 128 * (ic + 1)],
                        )
                else:
                    nc.sync.dma_start(out=w1dst, in_=w1src)
                if t == 0:
                    nc.sync.dma_start(
                        out=W2p[:].rearrange("q (h c) -> q h c", h=2),
                        in_=W2[:].rearrange("p o i -> p (o i)").rearrange(
                            "(h q) c -> q h c", q=128
                        ),
                    )

            # ---------------- DVE-side constant prep ----------------
            # Ordering matters: everything block 0 needs (band buffer 0,
            # W23T half 0, band13 half 0) is produced first so stage 1 can
            # start ~5us in; the rest fills DVE slack before its consumers.
            BW = 260 * NU + 4  # 2084: scatter view needs [pl4, pl4+2080)
            bufs = [
                pp.tile([128, BW], F32R, tag=f"band{i}", name=f"band{i}")
                for i in range(2)
            ]
            zsrc = pp.tile([128, 256], F32, tag="zsrc")
            nc.vector.memset(zsrc, 0.0)
            nc.vector.tensor_copy(
                bufs[0][:, 0:2048].rearrange("q (g c) -> q g c", c=256),
                zsrc[:, :].rearrange("q (g c) -> q g c", g=1).broadcast_to(
                    [128, 8, 256]
                ),
            )

            ident = pp.tile([128, 128], F32, tag="ident")
            make_identity(nc, ident)

            # W23 = W3@W2 per position on the DVE, W23T via PE transpose,
            # band13 (narrow W23^T block-diag) scatter -- per 128-pos half.
            MULT = mybir.AluOpType.mult
            AX_X = mybir.AxisListType.X
            W23p = pp.tile([128, 2 * O3 * O1], F32, tag="W23p")  # [128, 192]
            prod = pp.tile([128, O3 * O1 * O2], F32, tag="prod")  # [128, 768]
            W23T = pp.tile([32, NBLK * O3 * POS + 12], F32R, tag="W23T")
            band13 = pp.tile([128, 64 * 12 + 12], F32R, tag="band13")  # [128, 780]
            for h in range(2):
                prodv = prod[:, :].rearrange(
                    "q (x o1 o2) -> q x o1 o2", x=O3, o1=O1
                )
                nc.vector.tensor_tensor(
                    prodv,
                    w3p[:, 24 * h : 24 * (h + 1)]
                    .rearrange("q (x g o2) -> q x g o2", x=O3, g=1)
                    .broadcast_to([128, O3, O1, O2]),
                    W2p[:, 256 * h : 256 * (h + 1)]
                    .rearrange("q (g o2 o1) -> q g o1 o2", g=1, o2=O2)
                    .broadcast_to([128, O3, O1, O2]),
                    MULT,
                )
                nc.vector.tensor_reduce(
                    W23p[:, 96 * h : 96 * (h + 1)].rearrange(
                        "q (x o1) -> q x o1", x=O3
                    ),
                    prodv,
                    AX_X,
                    ADD,
                )
                # W23T[o1, 3p + x] = W23[p, x, o1] for this half
                psW = pstp.tile([128, 256], F32, tag="pst", name=f"psW{h}")
                nc.tensor.transpose(
                    psW[0:96, 0:128], W23p[:, 96 * h : 96 * (h + 1)],
                    ident[0:128, 0:128],
                )
                for x in range(O3):
                    dstw = (
                        W23T.bitcast(F32)[
                            0:32, 384 * h + x : 384 * h + x + 384
                        ].rearrange("q (c w) -> q c w", w=3)[:, :, 0]
                    )
                    nc.vector.tensor_copy(dstw, psW[32 * x : 32 * (x + 1), 0:128])
                if h == 0:
                    nc.vector.tensor_copy(
                        band13[:, 0:768].rearrange("q (g c) -> q g c", c=256),
                        zsrc[:, :].rearrange("q (g c) -> q g c", g=1).broadcast_to(
                            [128, 3, 256]
                        ),
                    )
                # band13[32 pl4 + o1, 12 g + 3 pl4 + x] = W23T[o1, 3(4g+pl4)+x]
                for pl4 in range(4):
                    dst13 = (
                        band13[
                            32 * pl4 : 32 * (pl4 + 1),
                            384 * h + 3 * pl4 : 384 * h + 3 * pl4 + 384,
                        ].rearrange("q (g w) -> q g w", w=12)[:, :, 0:O3]
                    )
                    src13 = (
                        W23T.bitcast(F32)[
                            :, 384 * h + 3 * pl4 : 384 * h + 3 * pl4 + 384
                        ].rearrange("q (g w) -> q g w", w=12)[:, :, 0:O3]
                    )
                    nc.vector.tensor_copy(dst13, src13)

            # glf [32, 512] -> glfT chunks: glfT[:, 32k:+32] = glf[:, 128k:+128].T
            glfT = pp.tile([128, 128], F32R, tag="glfT")
            for k in range(4):
                pt = pstp.tile([128, 256], F32, tag="pst", name=f"ptg{k}")
                nc.tensor.transpose(
                    pt[0:128, 0:B], glf_sb[:, 128 * k : 128 * (k + 1)],
                    ident[0:B, 0:B],
                )
                nc.vector.tensor_copy(glfT[:, 32 * k : 32 * k + 32], pt[0:128, 0:B])

            ones_sb = pp.tile([1, B], F32R, tag="ones")
            ones_f32 = pp.tile([1, B], F32, tag="ones32")
            nc.vector.memset(ones_f32, 1.0)
            nc.vector.tensor_copy(ones_sb, ones_f32)

            # b1_sb[q, g] = b1_flat[128 g + q] = transpose(b1B)
            b1_sb = pp.tile([128, 64 + 1], F32R, tag="b1")
            nc.vector.memset(b1_sb[:, 64:65].bitcast(F32), 0.0)
            ptb = pstp.tile([128, 256], F32, tag="pst", name="ptb")
            nc.tensor.transpose(
                ptb[0:128, 0:64], b1B[0:64, 0:128], ident[0:64, 0:64]
            )
            nc.vector.tensor_copy(b1_sb[:, 0:64], ptb[0:128, 0:64])

            # w3b2p[q, (h, x)] = sum_o2 W3[128h+q, x, o2] * b2[128h+q, o2]
            w3b2p = pp.tile([128, 2 * O3], F32, tag="w3b2p")  # [128, 6]
            prod2 = pp.tile([128, O3 * O2], F32, tag="prod2")  # [128, 24]
            for h in range(2):
                prod2v = prod2[:, :].rearrange("q (x o2) -> q x o2", x=O3)
                nc.vector.tensor_tensor(
                    prod2v,
                    w3p[:, 24 * h : 24 * (h + 1)].rearrange(
                        "q (x o2) -> q x o2", x=O3
                    ),
                    b2p[:, 8 * h : 8 * (h + 1)]
                    .rearrange("q (g o2) -> q g o2", g=1)
                    .broadcast_to([128, O3, O2]),
                    MULT,
                )
                nc.vector.tensor_reduce(
                    w3b2p[:, 3 * h : 3 * (h + 1)], prod2v, AX_X, ADD
                )

            # w3b2row[0, 96*blk + 32*x + pl] = (W3@b2)[32*blk + pl, x]
            w3b2row = pp.tile([1, NBLK * O3 * POS], F32, tag="w3b2")  # [1, 768]
            for h in range(2):
                for x in range(O3):
                    j = 3 * h + x
                    psj = pstp.tile([128, 256], F32, tag="pst", name=f"psj{j}")
                    nc.tensor.transpose(
                        psj[0:1, 0:128], w3b2p[:, j : j + 1], ident[0:128, 0:128]
                    )
                    dstj = w3b2row[0:1, :].rearrange(
                        "q (blk x pl) -> q blk x pl", x=O3, pl=POS
                    )[:, 4 * h : 4 * (h + 1), x, :]
                    srcj = psj[0:1, 0:128].rearrange(
                        "q (blk pl) -> q blk pl", pl=POS
                    )
                    nc.scalar.copy(dstj, srcj)

            # basebias = w3b2row + b3 (remapped flat (p,x) -> (blk,x,pl))
            basebias = pp.tile([1, NBLK * O3 * POS], F32, tag="bb")  # [1, 768]
            nc.vector.tensor_tensor(
                basebias[0:1, :].rearrange(
                    "q (blk x pl) -> q blk x pl", x=O3, pl=POS
                ),
                w3b2row[0:1, :].rearrange(
                    "q (blk x pl) -> q blk x pl", x=O3, pl=POS
                ),
                b3nat[0:1, :].rearrange(
                    "q (blk pl x) -> q blk x pl", pl=POS, x=O3
                ),
                ADD,
            )

            beffrow = pp.tile([1, NBLK * O3 * POS], F32R, tag="beff")  # [1, 768]
            out_sb = pp.tile([B, O3 * PP], F32, tag="outsb")  # [32, 768]

            nc.vector.tensor_copy(
                bufs[1][:, 0:2048].rearrange("q (g c) -> q g c", c=256),
                zsrc[:, :].rearrange("q (g c) -> q g c", g=1).broadcast_to(
                    [128, 8, 256]
                ),
            )

            # ---------------- main loop over position blocks ----------------
            # blocks 0-6: 32 positions on the two rotating big bands; the
            # final 32 positions run as two 16-position blocks on dedicated
            # small bands so almost no compute trails the last W1 byte.
            # Stage 2 of block b is emitted after stage 1 of block b+1
            # (software pipelining) so its matmuls never stall the PE on
            # the PSUM->SBUF drains.
            bufs16 = [
                pp.tile([128, 260 * 4 + 4], F32R, tag=f"b16_{i}", name=f"b16_{i}")
                for i in range(2)
            ]

            blocks = [(b, 32, bufs[b % 2]) for b in range(7)]
            blocks += [(7, 16, bufs16[0]), (8, 16, bufs16[1])]
            state = {}

            def emit_scatter(blk, POSb, buf):
                p0 = 32 * blk if POSb == 32 else 224 + 16 * (blk - 7)
                NUb = POSb // 4
                nb = 3 * POSb
                g0 = p0 // 4
                # scatter W23^T blocks into the sliding band
                for pl4 in range(4):
                    dst = (
                        buf[
                            32 * pl4 : 32 * (pl4 + 1),
                            pl4 : pl4 + 260 * NUb,
                        ].rearrange("q (u w) -> q u w", w=260)[:, :, 0:nb]
                        .rearrange("q u (x s) -> q u x s", s=POSb)[:, :, :, 0]
                    )
                    srcs = (
                        W23T.bitcast(F32)[
                            :, 3 * p0 + 3 * pl4 : 3 * p0 + 3 * pl4 + 12 * NUb
                        ].rearrange("q (u w) -> q u w", w=12)[:, :, 0:O3]
                    )
                    nc.vector.tensor_copy(dst, srcs)

            def emit_front(blk, POSb, buf):
                p0 = 32 * blk if POSb == 32 else 224 + 16 * (blk - 7)
                NUb = POSb // 4
                nb = 3 * POSb
                g0 = p0 // 4
                # stage 1: MeffT chunks, 4 i-chunks x NUb accumulating matmuls
                mts = []
                for c in range(4):
                    pst = pstp.tile(
                        [128, WIN], F32, tag="pst", name=f"pst{blk}_{c}"
                    )
                    for u in range(NUb):
                        w1t = w1tiles[(p0 + 4 * u) // 16]
                        lhsT = w1t[:].rearrange("q (v i) -> q v i", v=4)[
                            :, ((p0 + 4 * u) % 16) // 4, 128 * c : 128 * (c + 1)
                        ]
                        nc.tensor.matmul(
                            pst[:, 0:nb] if POSb == 16 else pst,
                            lhsT=lhsT,
                            rhs=buf[:, 256 * u : 256 * u + nb]
                            if POSb == 16
                            else buf[:, 256 * u : 256 * (u + 1)],
                            start=(u == 0),
                            stop=(u == NUb - 1),
                        )
                    mt = mtp.tile([128, WIN], F32R, tag="mt", name=f"mt{blk}_{c}")
                    mts.append(mt)
                    if c % 2 == 0:
                        nc.scalar.copy(mt[:, 0:nb], pst[:, 0:nb])
                    else:
                        nc.vector.tensor_copy(mt[:, 0:nb], pst[:, 0:nb])

                # beff part 1: W23@b1 via narrow band (NUb tiny matmuls)
                psb = psbp.tile([1, 96], F32, tag="psb", name=f"psb{blk}")
                for u in range(NUb):
                    g = g0 + u
                    nc.tensor.matmul(
                        psb[0:1, 12 * u : 12 * (u + 1)],
                        lhsT=b1_sb[:, g : g + 1],
                        rhs=band13[:, 12 * g : 12 * (g + 1)],
                        start=(u == 0),
                        stop=(u == NUb - 1),
                    )
                state[blk] = (POSb, p0, NUb, nb, psb, mts)

            def emit_tail(blk):
                POSb, p0, NUb, nb, psb, mts = state.pop(blk)
                half = 0 if POSb == 32 else blk - 7
                # beffrow_blk = remap(psb) + basebias_blk  (to (x, pl) layout)
                dstb = beffrow[0:1, 3 * p0 : 3 * p0 + nb].rearrange(
                    "q (x u pl4) -> q u pl4 x", x=O3, u=NUb
                )
                srcb = psb[0:1, 0 : 12 * NUb].rearrange(
                    "q (u pl4 x) -> q u pl4 x", u=NUb, x=O3
                )
                if POSb == 32:
                    srcc = basebias[0:1, 96 * blk : 96 * (blk + 1)].rearrange(
                        "q (x u pl4) -> q u pl4 x", x=O3, u=NUb
                    )
                else:
                    srcc = basebias[0:1, 672:768].rearrange(
                        "q (x hh u pl4) -> q hh u pl4 x", x=O3, hh=2, u=4
                    )[:, half]
                nc.vector.tensor_tensor(dstb, srcb, srcc, ADD)

                # stage 2: py[b, (x, pl)] = glf @ MeffT + beff
                py = psyp.tile([B, WIN], F32, tag="py", name=f"py{blk}")
                for c in range(3):
                    nc.tensor.matmul(
                        py[:, 0:nb],
                        lhsT=glfT[:, 32 * c : 32 * (c + 1)],
                        rhs=mts[c][:, 0:nb],
                        start=(c == 0),
                        stop=False,
                    )
                nc.tensor.matmul(
                    py[:, 0:nb],
                    lhsT=ones_sb,
                    rhs=beffrow[0:1, 3 * p0 : 3 * p0 + nb],
                    start=False,
                    stop=False,
                )
                nc.tensor.matmul(
                    py[:, 0:nb],
                    lhsT=glfT[:, 96:128],
                    rhs=mts[3][:, 0:nb],
                    start=False,
                    stop=True,
                )
                # drain py -> out_sb (global (x, p) layout)
                dsto = out_sb[:, :].rearrange("q (x p) -> q x p", x=O3)[
                    :, :, p0 : p0 + POSb
                ]
                nc.vector.tensor_copy(
                    dsto, py[:, 0:nb].rearrange("q (x pl) -> q x pl", x=O3)
                )
                if blk == 7:
                    # ship blocks 0..7a while the last block finishes
                    nc.scalar.dma_start(
                        out=out[:, :, 0:240],
                        in_=out_sb[:, :].rearrange("q (x p) -> q x p", x=O3)[
                            :, :, 0:240
                        ],
                    )
                elif blk == 8:
                    # last block on the idle sync queue -- shortest tail
                    nc.sync.dma_start(
                        out=out[:, :, 240:256],
                        in_=out_sb[:, :].rearrange("q (x p) -> q x p", x=O3)[
                            :, :, 240:256
                        ],
                    )

            emit_scatter(*blocks[0])
            for i, (blk, POSb, buf) in enumerate(blocks):
                if blk == 3:
                    # zero the small bands mid-stream: the DVE has slack
                    # here, and they are needed only from block 7 on
                    for bi in range(2):
                        nc.vector.tensor_copy(
                            bufs16[bi][:, 0:1024].rearrange(
                                "q (g c) -> q g c", c=256
                            ),
                            zsrc[:, :].rearrange(
                                "q (g c) -> q g c", g=1
                            ).broadcast_to([128, 4, 256]),
                        )
                if i + 1 < len(blocks):
                    # scatter for the NEXT block goes first in the DVE queue
                    # so this block's drains/py never delay it
                    emit_scatter(*blocks[i + 1])
                emit_front(blk, POSb, buf)
                if i > 0 and blocks[i - 1][0] in state:
                    emit_tail(blocks[i - 1][0])
                if POSb == 16:
                    emit_tail(blk)

    nc.compile()
    return nc


def _get_nc():
    if "nc" not in _CACHE:
        _CACHE["nc"] = _build_nc()
    return _CACHE["nc"]


def _make_in_maps(inputs):
    glf = np.ascontiguousarray(
        np.asarray(inputs["glf"], dtype=np.float32).reshape(B, I)
    )
    ins = {k: np.asarray(inputs[k], dtype=np.float32) for k in
           ("W1", "b1", "W2", "b2", "W3", "b3")}
    in_maps = []
    for c in range(NCORES):
        sl = slice(c * PP, (c + 1) * PP)
        in_maps.append(
            {
                "W1": np.ascontiguousarray(ins["W1"][sl]),
                "b1": np.ascontiguousarray(ins["b1"][sl]),
                "W2": np.ascontiguousarray(ins["W2"][sl]),
                "b2": np.ascontiguousarray(ins["b2"][sl]),
                "W3": np.ascontiguousarray(ins["W3"][sl]),
                "b3": np.ascontiguousarray(ins["b3"][sl]),
                "glf": glf,
            }
        )
    return in_maps


def run(inputs, trace=False):
    """Run on the 8 NeuronCores; returns (out_full, BassKernelResults)."""
    from concourse.bass_utils import run_bass_kernel_spmd

    nc = _get_nc()
    res = run_bass_kernel_spmd(
        nc, _make_in_maps(inputs), list(range(NCORES)), trace=trace
    )
    out_full = np.empty((B, O3, P_FULL), dtype=np.float32)
    for c in range(NCORES):
        out_full[:, :, c * PP : (c + 1) * PP] = res.results[c]["out"]
    return out_full, res


def kernel(**inputs):
    out, _ = run(inputs, trace=False)
    return out
